# revision 21
# baseline (speedup 1.0000x reference)
"""Trainium2 Bass kernel for nn_DeconvDft2dLayer.

Math reduction: w is [1, 8], so the padded filter hm1 occupies only row 0 of
the [H, W] grid. Hence fft2(hm1)[k, l] is independent of the row frequency k,
and the combined inverse-filter spectrum gmf[k, l] collapses to a real 1D
spectrum g1d[l] = |W1(l)|^-4 along W only (W1 = length-W FFT of the taps).
The H-axis FFT cancels with its inverse, so the whole layer is a per-row
circular convolution:

    y[b, h, :] = ifft(fft(x[b, h, :]) * g1d)  =  x[b, h, :] @ K

with K the real symmetric [W, W] circulant of ker = ifft(g1d), computed on
host from the 8 taps and replicated to all 8 cores; x is sharded over batch
(4 images per core).

Performance shape (per core): everything rides bf16 (absmax rel err ~4.4e-3
vs the 2e-2 gate; inputs are white noise so quantization error stays white
through the filter). IO drops to 2 MiB in + 2 MiB out + 0.25 MiB filter.
The circulant kernel decays fast enough that the far 128-block band
(|blockrow - blockcol| == 2 mod 4) contributes below bf16 noise, so each
128-row output chunk needs only 6 matmuls covering 1536 output columns
(vs 4x512 = 2048 for the full circulant). PSUM start=True clears the whole
bank's has_written bits, so partial-width accumulation windows compose.

Key tricks:
- K is shipped as its first 128 rows DUPLICATED side by side ([128, 2*W]):
  every circulant row-block j is a column rotation of block 0, and in the
  doubled buffer every rotated window is a CONTIGUOUS slice (rhs col
  p = q + (512 - 128j) mod 512). No on-device rotation copies (those made
  the Tile scheduler split chunks into two phases, serializing the tail).
- x is re-laid-out on host so every load group is one DMA with a single
  contiguous multi-KB run per partition; y is stored in SBUF-tile order
  (un-permuted on host) so stores are quad-chunk blocks with 4KB runs.
- Loads/stores are split across both HWDGE rings (SP=nc.sync,
  ACT=nc.scalar); a single queue caps at ~130-210 B/ns, two reach ~400.
- Dummy matmuls on an uninitialized scratch tile warm the PE's HAM clock
  gate (1.2 -> 2.4 GHz) during the initial DMA latency.
"""

import numpy as np
import ml_dtypes

import concourse.mybir as mybir
import concourse.tile as tile
from concourse import bacc, bass_utils

B, H, W = 32, 512, 512
N_CORES = 8
ROWS_PER_CORE = B * H // N_CORES  # 2048
N_CHUNKS = ROWS_PER_CORE // 128   # 16
# m-chunks per load group. Groups below 2 chunks are counterproductive:
# per-partition runs under 2KB drop the queue to ~140 B/ns (vs ~390), so a
# 1-chunk group completes LATER than a 2-chunk one.
GROUP_CHUNKS = (2, 2, 3, 3, 3, 3)
# ring per load, in issue order: [K, g0..g5]. K heads the SP ring (every
# chunk needs it); x group 0 heads the ACT ring.
LOAD_RINGS = ("sync", "scalar", "sync", "scalar", "sync", "scalar", "sync")
# (first_chunk, n_chunks, ring) per output store; quad stores have 4KB
# contiguous per-partition runs. Last chunks store singly (small tail).
STORE_PLAN = ((0, 4, "scalar"), (4, 4, "sync"), (8, 4, "scalar"),
              (12, 2, "sync"), (14, 1, "scalar"), (15, 1, "sync"))
N_WARM_MM = 6

# column windows per chunk: (block row j, src lo, src hi, out col lo), with
# src coords in the DOUBLED kt buffer [0, 2W): src p = q + (W - 128j) % W.
# Banded: kept output cols per block row j are blocks {j-1, j, j+1} mod 4.
# Ordered so the windows needing only kt[:, 0:W] come first (K half 1 lands
# ~0.7us before half 2).
WINDOWS_BANDED = (
    (0, 0, 256, 0), (0, 384, 512, 384), (3, 128, 256, 0),
    (1, 384, 768, 0), (2, 384, 768, 128), (3, 384, 640, 256),
)
WINDOWS_FULL = (
    (0, 0, 512, 0), (1, 384, 896, 0), (2, 256, 768, 0), (3, 128, 640, 0),
)

_nc_cache = None
LAST_RESULTS = None  # BassKernelResults of the most recent run (for test.py)


def _build(banded: bool):
    f32 = mybir.dt.float32
    bf16 = mybir.dt.bfloat16
    wins = WINDOWS_BANDED if banded else WINDOWS_FULL

    nc = bacc.Bacc("TRN2", target_bir_lowering=False, debug=False,
                   num_devices=N_CORES)
    # x shard, transposed + group-relaid on host: for load group g,
    # xt[:, 4*go : 4*(go+gc)] holds [p, (j, c)] = x[go + c, 128*j + p].
    xt_d = nc.dram_tensor("xt", [128, 4 * ROWS_PER_CORE], bf16,
                          kind="ExternalInput").ap()
    # first 128 circulant rows, duplicated: k[p, c] = K[p, c mod W]
    k_d = nc.dram_tensor("k", [128, 2 * W], bf16, kind="ExternalInput").ap()
    # y in SBUF-tile order: y_d[p, 512*i + q] = y[128*i + p, q]; host
    # un-permutes. Stores become plain 2D blocks with long runs.
    y_d = nc.dram_tensor("y", [128, N_CHUNKS * W], bf16,
                         kind="ExternalOutput").ap()

    # PE warm-up scratch: raw (untracked) SBUF, deliberately uninitialized —
    # the dummy matmul results are never read, and skipping a memset lets
    # the warm-up start as soon as the engines clear the entry barrier.
    scr = nc.alloc_sbuf_tensor("warm_scr", [128, W + 128], bf16).ap()

    group_cols = [128 * c for c in GROUP_CHUNKS]
    group_off = [128 * sum(GROUP_CHUNKS[:g]) for g in range(len(GROUP_CHUNKS))]

    with tile.TileContext(nc) as tc:
        with tc.tile_pool(name="const", bufs=1) as cpool, \
             tc.tile_pool(name="xtp", bufs=1) as xtpool, \
             tc.tile_pool(name="yout", bufs=6) as ypool, \
             tc.tile_pool(name="pyd", bufs=1, space="PSUM") as dpool, \
             tc.tile_pool(name="pyp", bufs=7, space="PSUM") as pypool:
            rings = {"sync": nc.sync, "scalar": nc.scalar}
            kt = cpool.tile([128, 2 * W], bf16, name="kt", tag="kt")
            rings[LOAD_RINGS[0]].dma_start(kt, k_d)

            # X^T resident in SBUF as one tile per load group, loads issued
            # up-front so they head the DMA sem-lane chains.
            xtgs = []
            for g, (gc, go) in enumerate(zip(group_cols, group_off)):
                t = xtpool.tile([128, 4 * gc], bf16, name=f"xtg{g}",
                                tag=f"xtg{g}")
                rings[LOAD_RINGS[g + 1]].dma_start(
                    t, xt_d[:, 4 * go:4 * (go + gc)])
                xtgs.append(t)

            # PE warm-up: dummy matmuls while the first loads are in flight.
            # HAM un-throttles the PE clock after ~3.4us of sustained busy;
            # these burn that window during DMA latency instead of on work.
            dummy = dpool.tile([128, W], f32, name="pyd", tag="pyd")
            for _ in range(N_WARM_MM):
                nc.tensor.matmul(dummy, scr[:, W:W + 128], scr[:, 0:W],
                                 start=True, stop=True)

            chunk_store = {}
            for s, (c0, nch, ring) in enumerate(STORE_PLAN):
                for c in range(nch):
                    chunk_store[c0 + c] = (s, c0, nch, ring)
            yo_tiles = {}

            for g, (nchunks, go) in enumerate(zip(GROUP_CHUNKS, group_off)):
                xtg = xtgs[g]
                gc = group_cols[g]
                for ci in range(nchunks):
                    i = go // 128 + ci
                    py = pypool.tile([128, W], f32, name=f"py{i}", tag="py")
                    for widx, (j, lo, hi, olo) in enumerate(wins):
                        lhsT = xtg[:, j * gc + 128 * ci:j * gc + 128 * (ci + 1)]
                        rhs = kt[:, lo:hi]
                        nc.tensor.matmul(
                            py[:, olo:olo + (hi - lo)], lhsT, rhs,
                            start=(widx == 0), stop=(widx == len(wins) - 1))
                    # copies (f32 PSUM -> bf16 SBUF cast) mostly on DVE; ACT
                    # also issues its ring's DMAs so it gets every third.
                    # NB: never split one chunk's copy across ACT+DVE —
                    # concurrent ScalarE+VectorE access to the SAME PSUM
                    # bank is a fatal HW collision.
                    copy_eng = (nc.scalar.copy if i % 3 == 1
                                else nc.vector.tensor_copy)
                    s, c0, nch, ring = chunk_store[i]
                    if s not in yo_tiles:
                        yo_tiles[s] = ypool.tile([128, nch * W], bf16,
                                                 name=f"yo{s}", tag=f"yo{s}",
                                                 bufs=1)
                    yo = yo_tiles[s]
                    copy_eng(yo[:, (i - c0) * W:(i - c0 + 1) * W], py)
                    if i == c0 + nch - 1:
                        rings[ring].dma_start(
                            y_d[:, c0 * W:(c0 + nch) * W], yo)

    nc.compile()
    return nc


def _filter_blocks(w: np.ndarray):
    """Doubled first circulant row-block (bf16) + banding safety check."""
    taps = np.asarray(w, np.float64).reshape(-1)
    W1 = np.fft.fft(np.pad(taps, (0, W - taps.shape[0])))
    g1d = 1.0 / (np.abs(W1) ** 4)
    ker = np.fft.ifft(g1d).real
    n = np.arange(W)
    K = ker[(n[None, :] - n[:, None]) % W]  # K[n, q] = ker[(q - n) mod W]

    # banding drops blocks (bj - bi) % 4 == 2; safe when the dropped mass is
    # well under the bf16 noise floor
    drop = 0.0
    for bi in range(4):
        bj = (bi + 2) % 4
        drop += np.linalg.norm(
            K[bi * 128:(bi + 1) * 128, bj * 128:(bj + 1) * 128]) ** 2
    banded = bool(np.sqrt(drop) / np.linalg.norm(K) < 5e-3)

    k0 = K[0:128].astype(np.float32).astype(ml_dtypes.bfloat16)
    kdbl = np.ascontiguousarray(np.concatenate([k0, k0], axis=1))
    return kdbl, banded


def _relayout_x(xshard: np.ndarray) -> np.ndarray:
    """[2048, 512] f32 -> [128, 8192] bf16 in load-group order."""
    xt = np.ascontiguousarray(xshard.T).astype(ml_dtypes.bfloat16)
    xt4 = xt.reshape(4, 128, ROWS_PER_CORE)
    segs = []
    off = 0
    for c in GROUP_CHUNKS:
        gc = 128 * c
        segs.append(np.ascontiguousarray(
            xt4[:, :, off:off + gc].transpose(1, 0, 2).reshape(128, 4 * gc)))
        off += gc
    return np.concatenate(segs, axis=1)


def kernel(x, w) -> np.ndarray:
    global _nc_cache, LAST_RESULTS
    kdbl, banded = _filter_blocks(np.asarray(w))
    if _nc_cache is None or _nc_cache[1] != banded:
        _nc_cache = (_build(banded), banded)
    nc = _nc_cache[0]

    xf = np.asarray(x, np.float32).reshape(N_CORES, ROWS_PER_CORE, W)
    in_maps = [{"xt": _relayout_x(xf[c]), "k": kdbl}
               for c in range(N_CORES)]
    res = bass_utils.run_bass_kernel_spmd(nc, in_maps,
                                          core_ids=list(range(N_CORES)))
    LAST_RESULTS = res
    # y_d[p, 512*i + q] = y[128*i + p, q]
    y = np.concatenate(
        [r["y"].astype(np.float32).reshape(128, N_CHUNKS, W)
         .transpose(1, 0, 2).reshape(ROWS_PER_CORE, W)
         for r in res.results], axis=0)
    return y.reshape(B, H, W, 1)
